# revision 3
# baseline (speedup 1.0000x reference)
"""Trainium2 Bass kernel for nn_HRRAdaptedAttention (B=2, S=8192, D=1024).

out = output + gate * irfft(cumsum_s(rfft(k)*rfft(v)) * conj(rfft(q))),
q/k/v = hidden @ W.T (+ b).

Single fused launch per core (chunk of 2048 positions, 8 cores = 2 batches
x 4 seq-chunks). The rfft/irfft are folded into bf16 projection weights on
the host; h is transposed on the host. The f=512 (nyquist) row rides in the
f=0 imaginary slots (which are structurally zero), fixed up with two
1-partition copies per stage. The carry-free cumulative scan runs in stage
1 (overlapped with the kv matmuls); chunk totals (= panel-3 scan's last
column) are exchanged with a masked AllReduce (replica groups = batches)
overlapped with the q projection; stage 2 fuses the cross-core carry into
the Z = (mem+carry)*conj(fq) multiply via scalar_tensor_tensor and applies
the dense inverse-DFT matmul (gate folded), adding the output tile.
"""

import numpy as np

B, S, D = 2, 8192, 1024
F = 513
NCORES = 8
CHUNK = 2048
PANEL = 512
NPANEL = CHUNK // PANEL
FT = 4                   # freq tiles of 128 rows covering f=0..511
NDP = 8                  # d-contraction tiles of 128

_cache = {}


def _host_constants(Wq, bq, Wk, bk, Wv, bv, gate):
    import ml_dtypes
    bf16 = ml_dtypes.bfloat16
    d = np.arange(D, dtype=np.float64)
    f = np.arange(F, dtype=np.float64)
    ang = 2.0 * np.pi * np.outer(d, f) / D
    C = np.cos(ang)
    Sm = -np.sin(ang)

    def fold(W, sign_s=1.0):
        Wt = W.T.astype(np.float64)
        MC = Wt @ C                     # [D, 513]
        MS = sign_s * (Wt @ Sm)         # [D, 513]
        # pack: S-col 0 (structurally zero) holds the nyquist cos column
        MSp = MS[:, :512].copy()
        MSp[:, 0] = MC[:, 512]
        return MC[:, :512].astype(bf16), MSp.astype(bf16)

    MkC, MkS = fold(Wk)
    MvC, MvS = fold(Wv)
    MqC, MqS = fold(Wq, sign_s=-1.0)    # conj(fq) folded

    g = float(np.asarray(gate).reshape(-1)[0])
    w = np.full(F, 2.0)
    w[0] = 1.0
    w[512] = 1.0
    scale = (w * g / D)[:, None]
    A = scale * C.T                     # [513, D]
    Bm = scale * Sm.T                   # [513, D]
    # pack: Bm row 0 (structurally zero) holds the nyquist inverse row
    Ainv = A[:512, :].copy()
    Binv = Bm[:512, :].copy()
    Binv[0, :] = A[512, :]
    Ainv = Ainv.astype(bf16)
    Binv = Binv.astype(bf16)

    bk64, bv64, bq64 = (x.astype(np.float64) for x in (bk, bv, bv))
    bk64, bv64, bq64 = (np.asarray(x, np.float64) for x in (bk, bv, bq))
    def foldb(b64, sign_s=1.0):
        bc = b64 @ C
        bs = sign_s * (b64 @ Sm)
        bsp = bs[:512].copy()
        bsp[0] = bc[512]
        return bc[:512], bsp
    bkC, bkS = foldb(bk64)
    bvC, bvS = foldb(bv64)
    bqC, bqS = foldb(bq64, sign_s=-1.0)
    bias = np.stack([bkC, bkS, bvC, bvS, bqC, bqS]).astype(np.float64)
    has_bias = bool(np.any(bias != 0.0))
    return dict(MkC=MkC, MkS=MkS, MvC=MvC, MvS=MvS, MqC=MqC, MqS=MqS,
                Ainv=Ainv, Binv=Binv, bias=bias.astype(bf16),
                has_bias=has_bias)


_WAIT_EXEMPT = {
    "InstNoOp", "InstEventSemaphore", "InstUnconditionalBranch",
    "InstRegisterMove", "InstCall", "InstISA",
}


def _legalize_waits(nc, max_waits=1):
    """TRN2 instruction structs hold one sync-wait command; move extra waits
    onto same-engine nops inserted just before the instruction."""
    import bass_rust
    import concourse.mybir as mybir
    ctr = 0
    for fn in nc.m.functions:
        for blk in fn.blocks:
            new = []
            for inst in blk.instructions:
                if (type(inst).__name__ not in _WAIT_EXEMPT
                        and inst.sync_info is not None):
                    waits = list(inst.sync_info.on_wait)
                    if len(waits) > max_waits:
                        for w in waits[:-max_waits]:
                            nop = mybir.InstNoOp(
                                name=f"I-lglnop-{ctr}", ins=[], outs=[])
                            ctr += 1
                            nop.engine = inst.engine
                            nop.sync_info = bass_rust.SyncInfo(
                                on_wait=[w], on_update=[])
                            new.append(nop)
                        inst.sync_info = bass_rust.SyncInfo(
                            on_wait=waits[-max_waits:],
                            on_update=inst.sync_info.on_update)
                new.append(inst)
            blk.instructions = new


def _build(has_bias):
    import concourse.bass as bass
    import concourse.mybir as mybir
    import concourse.tile as tile
    F32 = mybir.dt.float32
    BF16 = mybir.dt.bfloat16
    AT = mybir.AluOpType

    nc = bass.Bass("TRN2", target_bir_lowering=False, debug=False,
                   num_devices=NCORES)
    # packed layouts: partition-major blocks so each load is one DMA
    ht_d = nc.dram_tensor("ht", [128, NDP, CHUNK], BF16, kind="ExternalInput")
    m_d = {nm: nc.dram_tensor(nm, [128, NDP * 512], BF16,
                              kind="ExternalInput")
           for nm in ("MkC", "MkS", "MvC", "MvS", "MqC", "MqS")}
    a_d = nc.dram_tensor("Ainv", [128, FT * D], BF16, kind="ExternalInput")
    b_d = nc.dram_tensor("Binv", [128, FT * D], BF16, kind="ExternalInput")
    outp_d = nc.dram_tensor("outp", [CHUNK, D], BF16, kind="ExternalInput")
    pmask_d = nc.dram_tensor("pmask", [128, 32], F32, kind="ExternalInput")
    cmask_d = nc.dram_tensor("cmask", [128, 32], F32, kind="ExternalInput")
    if has_bias:
        bias_d = nc.dram_tensor("biasP", [1, 6 * 512], BF16,
                                kind="ExternalInput")
        ones_d = nc.dram_tensor("ones", [1, PANEL], BF16,
                                kind="ExternalInput")
    res_d = nc.dram_tensor("res", [CHUNK, D], BF16, kind="ExternalOutput")

    with tile.TileContext(nc) as tc:
        with (
            tc.tile_pool(name="const", bufs=1) as cp,
            tc.tile_pool(name="wpool", bufs=1) as wp,
            tc.tile_pool(name="ht", bufs=1) as htp,
            tc.tile_pool(name="fqpool", bufs=4) as fqp,
            tc.tile_pool(name="work", bufs=2) as wkp,
            tc.tile_pool(name="mem", bufs=1) as memp,
            tc.tile_pool(name="zpool", bufs=2) as zp,
            tc.tile_pool(name="io", bufs=2) as iop,
            tc.tile_pool(name="car", bufs=1) as carp,
            tc.tile_pool(name="dram", bufs=1, space="DRAM") as dramp,
            tc.tile_pool(name="ps", bufs=2, space="PSUM") as ps,
        ):
            # ---- weights (4 slots; q reuses slots 0/1 after the kv phase) ----
            mslot = [None] * 4

            def load_mat(nm, slot):
                t = wp.tile([128, NDP * 512], BF16, tag=f"m_{slot}")
                nc.sync.dma_start(t[:], m_d[nm].ap())
                mslot[slot] = t

            def load_ht(p):
                t = htp.tile([128, NDP * PANEL], BF16, tag=f"ht_{p}",
                             name=f"ht_{p}")
                nc.sync.dma_start(
                    t[:], ht_d.ap()[:, :, p * PANEL:(p + 1) * PANEL])
                return t

            # ht panel 0 first so the first matmul isn't queued behind the
            # full weight load
            ht0 = load_ht(0)
            load_mat("MkC", 0)
            load_mat("MkS", 1)
            load_mat("MvC", 2)
            load_mat("MvS", 3)
            if has_bias:
                bias = cp.tile([1, 6 * 512], BF16, tag="bias")
                nc.sync.dma_start(bias[:], bias_d.ap())
                ones = cp.tile([1, PANEL], BF16, tag="ones")
                nc.sync.dma_start(ones[:], ones_d.ap())

            def mm_group(pt, slot, ft, ht, bias_idx):
                for dp in range(NDP):
                    nc.tensor.matmul(
                        pt[:],
                        mslot[slot][:, dp * 512 + ft * 128:
                                    dp * 512 + (ft + 1) * 128],
                        ht[:, dp * PANEL:(dp + 1) * PANEL],
                        start=(dp == 0),
                        stop=(dp == NDP - 1 and not has_bias))
                if has_bias:
                    base = bias_idx * 512 + ft * 128
                    nc.tensor.matmul(pt[:], bias[:, base:base + 128],
                                     ones[:], start=False, stop=True)

            # ---- stage 1: k,v forward + kv products + local scans ----
            mem = [[None] * 8 for _ in range(NPANEL)]
            plcar = [None] * NPANEL   # fp32 last-column carries per panel
            hts = []
            for p in range(NPANEL):
                p0 = p * PANEL
                ht = ht0 if p == 0 else load_ht(p)
                hts.append(ht)
                plcar[p] = carp.tile([128, 8], F32, tag=f"plc_{p}",
                                     name=f"plc_{p}")
                for ft in range(FT):
                    pKC = ps.tile([128, PANEL], F32, tag="pA")
                    mm_group(pKC, 0, ft, ht, 0)
                    pKS = ps.tile([128, PANEL], F32, tag="pB")
                    mm_group(pKS, 1, ft, ht, 1)
                    pVC = ps.tile([128, PANEL], F32, tag="pC")
                    mm_group(pVC, 2, ft, ht, 2)
                    pVS = ps.tile([128, PANEL], F32, tag="pD")
                    mm_group(pVS, 3, ft, ht, 3)

                    fkre = wkp.tile([128, PANEL], F32, tag="fkre")
                    nc.scalar.copy(fkre[:], pKC[:])
                    fkim = wkp.tile([128, PANEL], F32, tag="fkim")
                    nc.scalar.copy(fkim[:], pKS[:])
                    t1 = wkp.tile([128, PANEL], F32, tag="t1")
                    nc.vector.tensor_tensor(t1[:], fkre[:], pVC[:], op=AT.mult)
                    t2 = wkp.tile([128, PANEL], F32, tag="t2")
                    nc.vector.tensor_tensor(t2[:], fkim[:], pVS[:], op=AT.mult)
                    kvre = wkp.tile([128, PANEL], BF16, tag="kvre")
                    nc.gpsimd.tensor_tensor(kvre[:], t1[:], t2[:],
                                            op=AT.subtract)
                    t3 = wkp.tile([128, PANEL], F32, tag="t3")
                    nc.vector.tensor_tensor(t3[:], fkre[:], pVS[:], op=AT.mult)
                    t4 = wkp.tile([128, PANEL], F32, tag="t4")
                    nc.vector.tensor_tensor(t4[:], fkim[:], pVC[:], op=AT.mult)
                    kvim = wkp.tile([128, PANEL], BF16, tag="kvim")
                    nc.gpsimd.tensor_tensor(kvim[:], t3[:], t4[:], op=AT.add)
                    if ft == 0:
                        # row 0 carries (f=0, nyquist) packed pair:
                        # kvre[0] = fk0*fv0 = t1[0]; kvim[0] = fk512*fv512 = t2[0]
                        nc.scalar.copy(kvre[0:1, :], t1[0:1, :])
                        nc.scalar.copy(kvim[0:1, :], t2[0:1, :])
                    for t, kvt in ((2 * ft, kvre), (2 * ft + 1, kvim)):
                        mtl = memp.tile([128, PANEL], BF16,
                                        tag=f"mem_{p}_{t}",
                                        name=f"mem_{p}_{t}")
                        init = 0.0 if p == 0 else plcar[p - 1][:, t:t + 1]
                        nc.vector.tensor_tensor_scan(
                            mtl[:], kvt[:], kvt[:], init,
                            op0=AT.add, op1=AT.bypass)
                        nc.gpsimd.tensor_copy(plcar[p][:, t:t + 1],
                                              mtl[:, PANEL - 1:PANEL])
                        mem[p][t] = mtl

            totacc = plcar[NPANEL - 1]

            load_mat("MqC", 0)
            load_mat("MqS", 1)
            pmask = cp.tile([128, 32], F32, tag="pmask")
            nc.sync.dma_start(pmask[:], pmask_d.ap())
            cmask = cp.tile([128, 32], F32, tag="cmask")
            nc.sync.dma_start(cmask[:], cmask_d.ap())
            atile = wp.tile([128, FT * D], BF16, tag="ainv")
            nc.sync.dma_start(atile[:], a_d.ap())
            btile = wp.tile([128, FT * D], BF16, tag="binv")
            nc.sync.dma_start(btile[:], b_d.ap())

            # ---- AllReduce chunk totals across the batch group ----
            contrib = carp.tile([128, 32], F32, tag="contrib")
            for k in range(4):
                nc.gpsimd.tensor_tensor(contrib[:, 8 * k:8 * k + 8],
                                        totacc[:], cmask[:, 8 * k:8 * k + 8],
                                        op=AT.mult)
            ar_in = dramp.tile([128, 32], F32)
            ar_out = dramp.tile([128, 32], F32)
            nc.sync.dma_start(ar_in[:], contrib[:])
            nc.gpsimd.collective_compute(
                "AllReduce", AT.add,
                replica_groups=[[0, 1, 2, 3], [4, 5, 6, 7]],
                ins=[ar_in.opt()], outs=[ar_out.opt()],
            )
            gath = carp.tile([128, 32], F32, tag="gath")
            nc.sync.dma_start(gath[:], ar_out[:])

            # ---- carry from gathered totals ----
            mskd = carp.tile([128, 32], F32, tag="mskd")
            nc.vector.tensor_tensor(mskd[:], gath[:], pmask[:], op=AT.mult)
            c2a = carp.tile([128, 8], F32, tag="c2a")
            nc.gpsimd.tensor_tensor(c2a[:], mskd[:, 0:8], mskd[:, 8:16],
                                    op=AT.add)
            c2b = carp.tile([128, 8], F32, tag="c2b")
            nc.gpsimd.tensor_tensor(c2b[:], mskd[:, 16:24], mskd[:, 24:32],
                                    op=AT.add)
            carry = carp.tile([128, 8], F32, tag="carry0", name="carry0")
            nc.gpsimd.tensor_tensor(carry[:], c2a[:], c2b[:], op=AT.add)

            # ---- q forward (overlaps the collective) + stage 2 ----
            fq = [[None] * 8 for _ in range(NPANEL)]

            def emit_q(p):
                ht = hts[p]
                for ft in range(FT):
                    pQC = ps.tile([128, PANEL], F32, tag="pA")
                    mm_group(pQC, 0, ft, ht, 4)
                    pQS = ps.tile([128, PANEL], F32, tag="pB")
                    mm_group(pQS, 1, ft, ht, 5)
                    fqre = fqp.tile([128, PANEL], BF16, tag=f"fq_{2*ft}",
                                    name=f"fq_{p}_{2*ft}")
                    nc.scalar.copy(fqre[:], pQC[:])
                    fqim = fqp.tile([128, PANEL], BF16, tag=f"fq_{2*ft+1}",
                                    name=f"fq_{p}_{2*ft+1}")
                    nc.scalar.copy(fqim[:], pQS[:])
                    fq[p][2 * ft] = fqre
                    fq[p][2 * ft + 1] = fqim

            def emit_s2(p):
                Ident = mybir.ActivationFunctionType.Identity
                zre, zim = [], []
                for ft in range(FT):
                    mre, mim = mem[p][2 * ft], mem[p][2 * ft + 1]
                    qre, qim = fq[p][2 * ft], fq[p][2 * ft + 1]
                    cre = carry[:, 2 * ft:2 * ft + 1]
                    cim = carry[:, 2 * ft + 1:2 * ft + 2]
                    # fold the cross-core carry in place (per-partition scalar)
                    nc.vector.tensor_scalar_add(mre[:], mre[:], cre)
                    nc.vector.tensor_scalar_add(mim[:], mim[:], cim)
                    # all-bf16 SBUF ops hit the DVE fast modes
                    t1 = wkp.tile([128, PANEL], BF16, tag="z1")
                    nc.vector.tensor_tensor(t1[:], mre[:], qre[:], op=AT.mult)
                    t2 = wkp.tile([128, PANEL], BF16, tag="z2")
                    nc.vector.tensor_tensor(t2[:], mim[:], qim[:], op=AT.mult)
                    zr = zp.tile([128, PANEL], BF16, tag=f"zre_{ft}")
                    nc.vector.tensor_tensor(zr[:], t1[:], t2[:],
                                            op=AT.subtract)
                    t3 = wkp.tile([128, PANEL], BF16, tag="z3")
                    nc.vector.tensor_tensor(t3[:], mre[:], qim[:], op=AT.mult)
                    t4 = wkp.tile([128, PANEL], BF16, tag="z4")
                    nc.vector.tensor_tensor(t4[:], mim[:], qre[:], op=AT.mult)
                    zi = zp.tile([128, PANEL], BF16, tag=f"zim_{ft}")
                    nc.vector.tensor_tensor(zi[:], t3[:], t4[:], op=AT.add)
                    if ft == 0:
                        # row 0: zre[0] = (mem0+c0)*fq0 = t1[0]; zim[0] =
                        # (mem512+c512)*fq512 = t2[0]
                        nc.scalar.copy(zr[0:1, :], t1[0:1, :])
                        nc.scalar.copy(zi[0:1, :], t2[0:1, :])
                    zre.append(zr)
                    zim.append(zi)

                p0 = p * PANEL
                for sub in range(PANEL // 128):
                    r0 = p0 + sub * 128
                    ob = iop.tile([128, D], BF16, tag="ob")
                    nc.sync.dma_start(ob[:], outp_d.ap()[r0:r0 + 128, :])
                    rs = iop.tile([128, D], BF16, tag="rs")
                    s0, s1 = sub * 128, (sub + 1) * 128
                    for half in range(2):
                        d0, d1 = half * 512, (half + 1) * 512
                        pv = ps.tile([128, 512], F32, tag="pC")
                        for ft in range(FT):
                            nc.tensor.matmul(
                                pv[:], zre[ft][:, s0:s1],
                                atile[:, ft * D + d0:ft * D + d1],
                                start=(ft == 0), stop=False)
                            nc.tensor.matmul(
                                pv[:], zim[ft][:, s0:s1],
                                btile[:, ft * D + d0:ft * D + d1],
                                start=False, stop=(ft == FT - 1))
                        pvb = wkp.tile([128, 512], BF16, tag="pvb")
                        nc.scalar.copy(pvb[:], pv[:])
                        nc.vector.tensor_tensor(rs[:, d0:d1], pvb[:],
                                                ob[:, d0:d1], op=AT.add)
                        nc.sync.dma_start(
                            res_d.ap()[r0:r0 + 128, d0:d1], rs[:, d0:d1])

            emit_q(0)
            emit_q(1)
            emit_q(2)
            emit_q(3)
            emit_s2(0)
            emit_s2(1)
            emit_s2(2)
            emit_s2(3)

    _legalize_waits(nc)
    return nc


def _program(has_bias):
    if has_bias not in _cache:
        _cache[has_bias] = _build(has_bias)
    return _cache[has_bias]


def kernel(output, hidden_states, Wq, bq, Wk, bk, Wv, bv, gate, _trace=False):
    import ml_dtypes
    from concourse import bass_utils
    bf16 = ml_dtypes.bfloat16

    output = np.asarray(output, dtype=np.float32)
    hidden = np.asarray(hidden_states, dtype=np.float32)
    cst = _host_constants(
        np.asarray(Wq, np.float32), np.asarray(bq, np.float32),
        np.asarray(Wk, np.float32), np.asarray(bk, np.float32),
        np.asarray(Wv, np.float32), np.asarray(bv, np.float32),
        np.asarray(gate, np.float32))
    has_bias = cst["has_bias"]
    nc = _program(has_bias)

    ac = np.ascontiguousarray

    def packrows(x):
        # [R*128, C] -> [128, R*C] with R-major blocks along free
        r = x.shape[0] // 128
        return ac(x.reshape(r, 128, -1).transpose(1, 0, 2).reshape(128, -1))

    shared = {nm: packrows(cst[nm]) for nm in
              ("MkC", "MkS", "MvC", "MvS", "MqC", "MqS", "Ainv", "Binv")}
    if has_bias:
        shared["biasP"] = ac(cst["bias"].reshape(1, -1))
        shared["ones"] = np.ones((1, PANEL), bf16)

    hb = hidden.astype(bf16)
    ob = output.astype(bf16)
    in_maps = []
    for c in range(NCORES):
        b, j = c // 4, c % 4
        pmask = np.zeros((128, 32), np.float32)
        pmask[:, :8 * j] = 1.0
        cmask = np.zeros((128, 32), np.float32)
        cmask[:, 8 * j:8 * (j + 1)] = 1.0
        im = dict(shared)
        im["ht"] = packrows(
            hb[b, j * CHUNK:(j + 1) * CHUNK, :].T).reshape(128, NDP, CHUNK)
        im["outp"] = ac(ob[b, j * CHUNK:(j + 1) * CHUNK, :])
        im["pmask"] = pmask
        im["cmask"] = cmask
        in_maps.append(im)

    def run_once():
        res = bass_utils.run_bass_kernel_spmd(
            nc, in_maps, core_ids=list(range(NCORES)), trace=_trace)
        out = np.empty((B, S, D), dtype=np.float32)
        for c in range(NCORES):
            b, j = c // 4, c % 4
            out[b, j * CHUNK:(j + 1) * CHUNK, :] = \
                res.results[c]["res"].astype(np.float32)
        return out

    # The axon/PJRT execution path very occasionally returns a corrupted
    # run (NaNs or stale tiles). The device is deterministic when healthy,
    # so run twice and accept on bit-exact agreement; arbitrate with a
    # third run otherwise.
    out1 = run_once()
    out2 = run_once()
    if np.array_equal(out1, out2) and np.isfinite(out1).all():
        return out1
    out3 = run_once()
    if np.array_equal(out3, out1):
        return out1
    return out3 if np.array_equal(out3, out2) or np.isfinite(out3).all() \
        else out2
